# revision 9
# baseline (speedup 1.0000x reference)
"""Trainium2 Bass kernel for 16-head MHA (B=2, S=2048, D=1024), fp32 I/O.

Sharding: tensor-parallel by heads across 8 NeuronCores. Core c owns heads
2c, 2c+1 (a 128-wide slice of the QKV projection output and of Wo's input
dim). Each core computes its head group's full attention plus a partial
output projection; the host sums the 8 partials.

Per-core dataflow (feature-major so the PE contraction dim is always the
SBUF partition dim; the host pre-transposes q/k/v and weights, and casts
activations + QKV weights to bf16):

  projections: TT = W_c @ x.T in [128, 1024]-column quarters, emitted
    interleaved with the first two attention windows so the input DMA
    (24 MB) overlaps scores/exp compute. V+ tiles ([128j, 64+1] with a
    ones column for the softmax denominator) come from VT via XBAR
    DMA-transpose -- no PE or PSUM involved.
  window pipeline (4 windows of 1024 query cols, 16 j-chunk positions,
  6-deep loop; three stages overlap):
    scores(L):  S.T [128j, 1024i] = KT.T @ QT per head -> exp. exp runs
                on ACT (activation Exp) or DVE via the Schraudolph
                identity bf16(exp(s/8)) == bitcast<u16>(s*23.083+16250.5)
                (f32->u16 saturates negatives to 0 == exp underflow;
                ~3.3% max rel err). Engine checkerboard (2p+h)%3 keeps
                both engines below the PE's per-position cadence.
    av(L-1):    O+ [65, 1024] += V+.T @ E, start-delayed 2 positions
                (8 positions in loop 1, which waits out the input DMA).
    normalize(L-2): denom -> reciprocal_approx_fast, replicate across
                head dims via gpsimd partition_broadcast, OC = O+ * rep.
    wo(L-2):    out rows = OC.T @ WoT_c (fp32r), positions 4..11,
                DVE PSUM->SBUF copy, DMA out.
"""

import sys

sys.path.insert(0, "/opt/trn_rl_repo")

import numpy as np

import concourse.bacc as bacc
import concourse.mybir as mybir
import concourse.tile as tile
from concourse.bass_utils import run_bass_kernel_spmd

F32 = mybir.dt.float32
R = mybir.dt.float32r
BF16 = mybir.dt.bfloat16
U16 = mybir.dt.uint16
EXP = mybir.ActivationFunctionType.Exp
MULT = mybir.AluOpType.mult
ADD = mybir.AluOpType.add

D = 1024
BATCH = 2
SEQ = 2048
M = BATCH * SEQ  # 4096 token rows
HEADS_PER_CORE = 2
DK = 64
HG = HEADS_PER_CORE * DK  # 128-wide head-group slice per core
N_CORES = 8
KT_TILES = D // 128  # 8 contraction tiles for the projections
JC = SEQ // 128  # 16 j-chunks per batch
N_WIN = 4  # (b, ih) windows of 1024 query columns
SCALE = 1.0 / np.sqrt(DK)

# Schraudolph exp in bf16 bits, C=5.5 tuned offline for min max-rel-err
SCH_A = float(np.log2(np.e) * 128.0) * SCALE
SCH_B = 16256.0 - 5.5


def build_bass():
    nc = bacc.Bacc(None)

    qT = nc.dram_tensor("qT", [D, M], BF16, kind="ExternalInput")
    kT = nc.dram_tensor("kT", [D, M], BF16, kind="ExternalInput")
    vT = nc.dram_tensor("vT", [D, M], BF16, kind="ExternalInput")
    wqT = nc.dram_tensor("wqT", [D, HG], BF16, kind="ExternalInput")
    wkT = nc.dram_tensor("wkT", [D, HG], BF16, kind="ExternalInput")
    wvT = nc.dram_tensor("wvT", [D, HG], BF16, kind="ExternalInput")
    woT = nc.dram_tensor("woT", [HG, D], R, kind="ExternalInput")
    out = nc.dram_tensor("out", [M, D], F32, kind="ExternalOutput")

    dma_ctr = [0]

    def dma_eng():
        dma_ctr[0] += 1
        return nc.sync if dma_ctr[0] % 2 == 0 else nc.scalar

    with tile.TileContext(nc) as tc:
        with (
            tc.tile_pool(name="consts", bufs=1) as cst,
            tc.tile_pool(name="acts", bufs=1) as acts,
            tc.tile_pool(name="vp", bufs=1) as vp_pool,
            tc.tile_pool(name="ocpool", bufs=2) as ocpool,
            tc.tile_pool(name="outpool", bufs=2) as outpool,
            tc.tile_pool(name="small", bufs=1) as small,
            tc.tile_pool(name="epool", bufs=52) as epool,
            tc.tile_pool(name="psb", bufs=2, space="PSUM") as psb,
        ):
            ones_f = cst.tile([128, 1], F32)
            nc.gpsimd.memset(ones_f[:], 1.0)
            # warm the ACT exp table while DMA streams inputs
            scratch = cst.tile([1, 64], F32)
            nc.scalar.activation(
                scratch[:], ones_f[0:1, 0:1].to_broadcast([1, 64]), EXP
            )

            wo_sb = acts.tile([HG, D], R)
            nc.sync.dma_start(wo_sb[:], woT[:])

            QT = acts.tile([HG, M], BF16)
            KT = acts.tile([HG, M], BF16)
            VT = acts.tile([HG, M], BF16)

            vp_tiles = {}
            windows = [(b, ih) for b in range(BATCH) for ih in range(2)]

            def emit_scores(st, p):
                b, ih = st["w"]
                i0 = b * SEQ + ih * 1024
                j0 = b * SEQ + p * 128
                for h in range(HEADS_PER_CORE):
                    hs = slice(h * DK, (h + 1) * DK)
                    ps_s = psb.tile([128, 1024], F32, tag="big")
                    for iw in range(2):
                        nc.tensor.matmul(
                            ps_s[:, iw * 512 : (iw + 1) * 512],
                            KT[hs, j0 : j0 + 128],
                            QT[hs, i0 + iw * 512 : i0 + (iw + 1) * 512],
                            start=True,
                            stop=True,
                        )
                    e_t = epool.tile([128, 1024], BF16, tag="e")
                    if (2 * p + h) % 3 == 2:
                        nc.vector.tensor_scalar(
                            e_t[:].bitcast(U16), ps_s[:], SCH_A, SCH_B, MULT, ADD
                        )
                    else:
                        nc.scalar.activation(e_t[:], ps_s[:], EXP, scale=SCALE)
                    st["e"][(h, p)] = e_t

            def emit_av(st, pso, chunks):
                b = st["w"][0]
                for jc in chunks:
                    if jc == 0:
                        st["po"] = {
                            h: pso.tile(
                                [DK + 1, 1024], F32, tag="po", name=f"po{h}"
                            )
                            for h in range(HEADS_PER_CORE)
                        }
                    po, e_tiles = st["po"], st["e"]
                    jg = b * JC + jc
                    for h in range(HEADS_PER_CORE):
                        for iw in range(2):
                            nc.tensor.matmul(
                                po[h][:, iw * 512 : (iw + 1) * 512],
                                vp_tiles[(h, jg)][:],
                                e_tiles[(h, jc)][:, iw * 512 : (iw + 1) * 512],
                                start=(jc == 0),
                                stop=(jc == JC - 1),
                            )

            def emit_normalize(st):
                po = st["po"]
                oc = ocpool.tile([HG, 1024], R, tag="oc")
                for h in range(HEADS_PER_CORE):
                    hs = slice(h * DK, (h + 1) * DK)
                    dn = small.tile([1, 1024], F32, tag=f"dn{h}", name=f"dn{h}")
                    nc.vector.tensor_copy(dn[:], po[h][DK : DK + 1, :])
                    rr = small.tile([1, 1024], F32, tag=f"rr{h}", name=f"rr{h}")
                    nc.vector.reciprocal_approx_fast(rr[:], dn[:])
                    rb = small.tile([64, 1024], F32, tag=f"rb{h}", name=f"rb{h}")
                    nc.gpsimd.partition_broadcast(rb[:], rr[0:1, :])
                    nc.vector.tensor_tensor(oc[hs, :], po[h][0:DK, :], rb[:], MULT)
                st["oc"] = oc

            def emit_wo(st, ic):
                b, ih = st["w"]
                oc = st["oc"]
                i0 = b * SEQ + ih * 1024
                wo_ps = psb.tile([128, 1024], F32, tag="big")
                for oh in range(2):
                    nc.tensor.matmul(
                        wo_ps[:, oh * 512 : (oh + 1) * 512],
                        oc[:, ic * 128 : (ic + 1) * 128],
                        wo_sb[:, oh * 512 : (oh + 1) * 512],
                        start=True,
                        stop=True,
                    )
                out_sb = outpool.tile([128, 1024], F32, tag="os")
                nc.vector.tensor_copy(out_sb[:], wo_ps[:])
                r0 = i0 + ic * 128
                dma_eng().dma_start(out[r0 : r0 + 128, :], out_sb[:])

            # av start-delay maps
            AV_CHUNKS = {p: [] for p in range(JC)}
            for p in range(2, 14):
                AV_CHUNKS[p].append(p - 2)
            AV_CHUNKS[14] = [12, 13]
            AV_CHUNKS[15] = [14, 15]
            AV1_CHUNKS = {p: [] for p in range(JC)}
            for p in range(8, 16):
                AV1_CHUNKS[p] = [2 * (p - 8), 2 * (p - 8) + 1]
            WO_POS = {p: p - 4 for p in range(4, 12)}

            with (
                tc.tile_pool(name="wpool", bufs=1) as wpool,
                tc.tile_pool(name="stage", bufs=3) as stage,
                tc.tile_pool(name="pp", bufs=2, space="PSUM") as pp,
            ):
                wq_sb = wpool.tile([128, KT_TILES, HG], BF16)
                wk_sb = wpool.tile([128, KT_TILES, HG], BF16)
                wv_sb = wpool.tile([128, KT_TILES, HG], BF16)
                for w_sb, w_dram in ((wk_sb, wkT), (wq_sb, wqT), (wv_sb, wvT)):
                    nc.sync.dma_start(
                        w_sb[:], w_dram.rearrange("(ko p) n -> p ko n", p=128)
                    )

                def emit_quarter_kstep(TT, w_sb, x_dram, base, k, pq_box):
                    if k == 0:
                        pq_box[0] = pp.tile([128, 1024], F32, tag="pq", name="pq")
                    pq = pq_box[0]
                    xst = stage.tile([128, 1024], BF16, tag="xst")
                    dma_eng().dma_start(
                        xst[:], x_dram[k * 128 : (k + 1) * 128, base : base + 1024]
                    )
                    for nh in range(2):
                        nc.tensor.matmul(
                            pq[:, nh * 512 : (nh + 1) * 512],
                            w_sb[:, k, :],
                            xst[:, nh * 512 : (nh + 1) * 512],
                            start=(k == 0),
                            stop=(k == KT_TILES - 1),
                        )
                    if k == KT_TILES - 1:
                        nc.vector.tensor_copy(TT[:, base : base + 1024], pq[:])

                def emit_vplus(b):
                    for h in range(HEADS_PER_CORE):
                        hs = slice(h * DK, (h + 1) * DK)
                        for j16 in range(JC):
                            jg = b * JC + j16
                            vpt = vp_pool.tile(
                                [128, DK + 1], BF16, tag=f"vp_{h}_{jg}"
                            )
                            nc.gpsimd.memset(vpt[:, DK : DK + 1], 1.0)
                            dma_eng().dma_start(
                                vpt[:, 0:DK],
                                VT[hs, jg * 128 : (jg + 1) * 128],
                                transpose=True,
                            )
                            vp_tiles[(h, jg)] = vpt

                # quarter order: K_b0, Q_b0.q0 pre-emitted; rest interleaved
                quarters = []
                for b in range(BATCH):
                    for TT, w_sb, x_dram, t in (
                        (KT, wk_sb, kT, "k"),
                        (QT, wq_sb, qT, "q"),
                        (VT, wv_sb, vT, "v"),
                    ):
                        for q in range(2):
                            quarters.append((TT, w_sb, x_dram, t, b, q))
                # reorder: K b0 (2), Q b0 (2), V b0 (2), K b1, Q b1, V b1
                steps = []
                for TT, w_sb, x_dram, t, b, q in quarters:
                    base = b * 2048 + q * 1024
                    box = [None]
                    for k in range(KT_TILES):
                        steps.append(
                            (
                                lambda TT=TT, w_sb=w_sb, x_dram=x_dram,
                                base=base, k=k, box=box: emit_quarter_kstep(
                                    TT, w_sb, x_dram, base, k, box
                                )
                            )
                        )
                    if t == "v" and q == 1:
                        steps.append(lambda b=b: emit_vplus(b))

                # pre-emit: K_b0 both quarters + Q_b0 quarter 0 (24 k-steps)
                for s in steps[:24]:
                    s()
                rest = steps[24:]
                ri = 0

                def run_steps(n):
                    nonlocal ri
                    for _ in range(n):
                        if ri < len(rest):
                            rest[ri]()
                            ri += 1

                # loop 0: scores(w0) + interleaved projection steps
                st0 = {"w": windows[0], "e": {}}
                for p in range(JC):
                    emit_scores(st0, p)
                    run_steps(4)

                # loop 1 first half: scores(w1) + remaining projections
                st1 = {"w": windows[1], "e": {}}
                for p in range(8):
                    emit_scores(st1, p)
                    run_steps(3)
                run_steps(len(rest))  # flush any stragglers

            # projection pools closed; attention-only phase
            with tc.tile_pool(name="pso", bufs=2, space="PSUM") as pso:
                # loop 1 second half: scores(w1) + av(w0) catch-up
                for p in range(8, JC):
                    emit_scores(st1, p)
                    emit_av(st0, pso, AV1_CHUNKS[p])

                av_st, wo_st = st1, st0
                for L in range(2, N_WIN + 2):
                    sc_st = (
                        {"w": windows[L], "e": {}} if L < N_WIN else None
                    )
                    if wo_st is not None:
                        emit_normalize(wo_st)
                    for p in range(JC):
                        if sc_st is not None:
                            emit_scores(sc_st, p)
                        if av_st is not None:
                            emit_av(av_st, pso, AV_CHUNKS[p])
                        if wo_st is not None and p in WO_POS:
                            emit_wo(wo_st, WO_POS[p])
                    wo_st, av_st = av_st, sc_st

    nc.compile()
    return nc


def kernel(q, k, v, Wq, Wk, Wv, Wo):
    import ml_dtypes

    bf = ml_dtypes.bfloat16
    q = np.asarray(q, dtype=np.float32)
    k = np.asarray(k, dtype=np.float32)
    v = np.asarray(v, dtype=np.float32)
    Wq = np.asarray(Wq, dtype=np.float32)
    Wk = np.asarray(Wk, dtype=np.float32)
    Wv = np.asarray(Wv, dtype=np.float32)
    Wo = np.asarray(Wo, dtype=np.float32)

    qT = np.ascontiguousarray(q.reshape(M, D).T.astype(bf))
    kT = np.ascontiguousarray(k.reshape(M, D).T.astype(bf))
    vT = np.ascontiguousarray(v.reshape(M, D).T.astype(bf))

    in_maps = []
    for c in range(N_CORES):
        cs = slice(c * HG, (c + 1) * HG)
        in_maps.append(
            {
                "qT": qT,
                "kT": kT,
                "vT": vT,
                "wqT": np.ascontiguousarray(Wq[cs, :].T.astype(bf)),
                "wkT": np.ascontiguousarray(Wk[cs, :].T.astype(bf)),
                "wvT": np.ascontiguousarray(Wv[cs, :].T.astype(bf)),
                "woT": np.ascontiguousarray(Wo[:, cs].T),
            }
        )

    nc = build_bass()

    def run_once():
        res = run_bass_kernel_spmd(nc, in_maps, core_ids=list(range(N_CORES)))
        acc = res.results[0]["out"].astype(np.float32)
        for c in range(1, N_CORES):
            acc = acc + res.results[c]["out"]
        return acc

    acc = run_once()
    if not np.isfinite(acc).all():
        acc = run_once()  # guard against sporadic device flake
    return acc.reshape(BATCH, SEQ, D)


# revision 11
# speedup vs baseline: 1.3598x; 1.3598x over previous
"""Trainium2 Bass kernel for 16-head MHA (B=2, S=2048, D=1024), fp32 I/O.

Sharding: tensor-parallel by heads across 8 NeuronCores. Core c owns heads
2c, 2c+1 (a 128-wide slice of the QKV projection output and of Wo's input
dim). Each core computes its head group's full attention plus a partial
output projection; the host sums the 8 partials.

Per-core dataflow (feature-major so the PE contraction dim is always the
SBUF partition dim; the host pre-transposes q/k/v and weights and casts
to fp16 -- same PE speed as bf16, 8x finer mantissa):

  projections: TT = W_c @ x.T streamed as half-batch k-tiles
    [128, 2048] (4 KB DMA lines, all on the sync queue), accumulated in
    one [128, 2048] PSUM tile, emitted interleaved with the first two
    attention windows so the 24 MB input DMA overlaps scores/exp.
  V+ tiles ([128j, 16jc, 64+1] per (head, batch), the 65th column ones
    for the softmax denominator): one XBAR DMA-transpose per (head,
    batch) into a contiguous staging tile + a strided DVE copy.
  window pipeline (4 windows of 1024 query cols, 16 j-chunk positions,
  three stages overlap):
    scores(L):  S.T [128j, 1024i] = KT.T @ QT per head -> exp on ACT
                (activation Exp) or DVE via the Schraudolph identity
                fp16(exp(s/8)) == bitcast<u16>(s*184.66*SCALE + 15315)
                (f32->u16 saturates negatives to 0 == exp underflow;
                ~3% max rel err on its share of weights). Checkerboard
                (2p+h)%3 keeps both engines under the PE cadence.
    av(L-1):    O+ [65, 1024] += V+.T @ E, start-delayed 2 positions
                (8 positions in loop 1, which waits out the input DMA).
    normalize(L-2): reciprocal_approx_fast on the PSUM denominator row,
                replicate via gpsimd partition_broadcast, OC = O+ * rep.
    wo(L-2):    out rows = OC.T @ WoT_c (fp32r), positions 4..11, DVE
                PSUM->SBUF copy, out DMA on sync.
"""

import sys

sys.path.insert(0, "/opt/trn_rl_repo")

import numpy as np

import concourse.bacc as bacc
import concourse.mybir as mybir
import concourse.tile as tile
from concourse.bass_utils import run_bass_kernel_spmd

F32 = mybir.dt.float32
R = mybir.dt.float32r
F16 = mybir.dt.float16
U16 = mybir.dt.uint16
EXP = mybir.ActivationFunctionType.Exp
MULT = mybir.AluOpType.mult
ADD = mybir.AluOpType.add

D = 1024
BATCH = 2
SEQ = 2048
M = BATCH * SEQ  # 4096 token rows
HEADS_PER_CORE = 2
DK = 64
HG = HEADS_PER_CORE * DK  # 128-wide head-group slice per core
N_CORES = 8
KT_TILES = D // 128  # 8 contraction tiles for the projections
JC = SEQ // 128  # 16 j-chunks per batch
N_WIN = 4  # (b, ih) windows of 1024 query columns
SCALE = 1.0 / np.sqrt(DK)

# Schraudolph exp in fp16 bits, C=45 tuned offline for min max-rel-err (3.0%)
SCH_A = float(np.log2(np.e) * 1024.0) * SCALE
SCH_B = 15360.0 - 45.0


def build_bass():
    nc = bacc.Bacc(None)

    qT = nc.dram_tensor("qT", [D, M], F16, kind="ExternalInput")
    kT = nc.dram_tensor("kT", [D, M], F16, kind="ExternalInput")
    vT = nc.dram_tensor("vT", [D, M], F16, kind="ExternalInput")
    wqT = nc.dram_tensor("wqT", [D, HG], F16, kind="ExternalInput")
    wkT = nc.dram_tensor("wkT", [D, HG], F16, kind="ExternalInput")
    wvT = nc.dram_tensor("wvT", [D, HG], F16, kind="ExternalInput")
    woT = nc.dram_tensor("woT", [HG, D], R, kind="ExternalInput")
    out = nc.dram_tensor("out", [M, D], F32, kind="ExternalOutput")

    with tile.TileContext(nc) as tc:
        with (
            tc.tile_pool(name="consts", bufs=1) as cst,
            tc.tile_pool(name="acts", bufs=1) as acts,
            tc.tile_pool(name="vp", bufs=1) as vp_pool,
            tc.tile_pool(name="ocpool", bufs=2) as ocpool,
            tc.tile_pool(name="outpool", bufs=2) as outpool,
            tc.tile_pool(name="small", bufs=1) as small,
            tc.tile_pool(name="epool", bufs=52) as epool,
            tc.tile_pool(name="psb", bufs=2, space="PSUM") as psb,
        ):
            ones_f = cst.tile([128, 1], F32)
            nc.gpsimd.memset(ones_f[:], 1.0)
            # warm the ACT exp table while DMA streams inputs
            scratch = cst.tile([1, 64], F32)
            nc.scalar.activation(
                scratch[:], ones_f[0:1, 0:1].to_broadcast([1, 64]), EXP
            )

            wo_sb = acts.tile([HG, D], R)
            nc.sync.dma_start(wo_sb[:], woT[:])

            QT = acts.tile([HG, M], F16)
            KT = acts.tile([HG, M], F16)
            VT = acts.tile([HG, M], F16)

            vp_tiles = {}
            windows = [(b, ih) for b in range(BATCH) for ih in range(2)]

            def emit_scores(st, p):
                b, ih = st["w"]
                i0 = b * SEQ + ih * 1024
                j0 = b * SEQ + p * 128
                for h in range(HEADS_PER_CORE):
                    hs = slice(h * DK, (h + 1) * DK)
                    ps_s = psb.tile([128, 1024], F32, tag="big")
                    for iw in range(2):
                        nc.tensor.matmul(
                            ps_s[:, iw * 512 : (iw + 1) * 512],
                            KT[hs, j0 : j0 + 128],
                            QT[hs, i0 + iw * 512 : i0 + (iw + 1) * 512],
                            start=True,
                            stop=True,
                        )
                    e_t = epool.tile([128, 1024], F16, tag="e")
                    if (2 * p + h) % 3 == 2:
                        nc.vector.tensor_scalar(
                            e_t[:].bitcast(U16), ps_s[:], SCH_A, SCH_B, MULT, ADD
                        )
                    else:
                        nc.scalar.activation(e_t[:], ps_s[:], EXP, scale=SCALE)
                    st["e"][(h, p)] = e_t

            def emit_av(st, pso, chunks):
                b = st["w"][0]
                for jc in chunks:
                    if jc == 0:
                        st["po"] = {
                            h: pso.tile(
                                [DK + 1, 1024], F32, tag="po", name=f"po{h}"
                            )
                            for h in range(HEADS_PER_CORE)
                        }
                    po, e_tiles = st["po"], st["e"]
                    for h in range(HEADS_PER_CORE):
                        for iw in range(2):
                            nc.tensor.matmul(
                                po[h][:, iw * 512 : (iw + 1) * 512],
                                vp_tiles[(h, b)][:, jc, :],
                                e_tiles[(h, jc)][:, iw * 512 : (iw + 1) * 512],
                                start=(jc == 0),
                                stop=(jc == JC - 1),
                            )

            def emit_normalize(st):
                po = st["po"]
                oc = ocpool.tile([HG, 1024], R, tag="oc")
                for h in range(HEADS_PER_CORE):
                    hs = slice(h * DK, (h + 1) * DK)
                    dn = small.tile([1, 1024], F32, tag=f"dn{h}", name=f"dn{h}")
                    nc.vector.tensor_copy(dn[:], po[h][DK : DK + 1, :])
                    rr = small.tile([1, 1024], F32, tag=f"rr{h}", name=f"rr{h}")
                    nc.vector.reciprocal_approx_fast(rr[:], dn[:])
                    rb = small.tile([64, 1024], F32, tag=f"rb{h}", name=f"rb{h}")
                    nc.gpsimd.partition_broadcast(rb[:], rr[0:1, :])
                    nc.vector.tensor_tensor(oc[hs, :], po[h][0:DK, :], rb[:], MULT)
                st["oc"] = oc

            def emit_wo(st, ic):
                b, ih = st["w"]
                oc = st["oc"]
                i0 = b * SEQ + ih * 1024
                wo_ps = psb.tile([128, 1024], F32, tag="big")
                for oh in range(2):
                    nc.tensor.matmul(
                        wo_ps[:, oh * 512 : (oh + 1) * 512],
                        oc[:, ic * 128 : (ic + 1) * 128],
                        wo_sb[:, oh * 512 : (oh + 1) * 512],
                        start=True,
                        stop=True,
                    )
                out_sb = outpool.tile([128, 1024], F32, tag="os")
                nc.vector.tensor_copy(out_sb[:], wo_ps[:])
                r0 = i0 + ic * 128
                nc.sync.dma_start(out[r0 : r0 + 128, :], out_sb[:])

            AV_CHUNKS = {p: [] for p in range(JC)}
            for p in range(2, 14):
                AV_CHUNKS[p].append(p - 2)
            AV_CHUNKS[14] = [12, 13]
            AV_CHUNKS[15] = [14, 15]
            AV1_CHUNKS = {p: [2 * (p - 8), 2 * (p - 8) + 1] for p in range(8, 16)}
            WO_POS = {p: p - 4 for p in range(4, 12)}

            with (
                tc.tile_pool(name="wpool", bufs=1) as wpool,
                tc.tile_pool(name="stage", bufs=3) as stage,
                tc.tile_pool(name="pp", bufs=1, space="PSUM") as pp,
            ):
                wq_sb = wpool.tile([128, KT_TILES, HG], F16)
                wk_sb = wpool.tile([128, KT_TILES, HG], F16)
                wv_sb = wpool.tile([128, KT_TILES, HG], F16)
                for w_sb, w_dram in ((wk_sb, wkT), (wq_sb, wqT), (wv_sb, wvT)):
                    nc.scalar.dma_start(
                        w_sb[:], w_dram.rearrange("(ko p) n -> p ko n", p=128)
                    )

                def emit_half_kstep(TT, w_sb, x_dram, b, k, pq_box):
                    base = b * 2048
                    if k == 0:
                        pq_box[0] = pp.tile(
                            [128, 2048], F32, tag="pq", name="pq"
                        )
                    pq = pq_box[0]
                    xst = stage.tile([128, 2048], F16, tag="xst", name="xst")
                    nc.sync.dma_start(
                        xst[:], x_dram[k * 128 : (k + 1) * 128, base : base + 2048]
                    )
                    for nh in range(4):
                        nc.tensor.matmul(
                            pq[:, nh * 512 : (nh + 1) * 512],
                            w_sb[:, k, :],
                            xst[:, nh * 512 : (nh + 1) * 512],
                            start=(k == 0),
                            stop=(k == KT_TILES - 1),
                        )
                    if k == KT_TILES - 1:
                        nc.vector.tensor_copy(TT[:, base : base + 2048], pq[:])

                def emit_vplus(b):
                    for h in range(HEADS_PER_CORE):
                        hs = slice(h * DK, (h + 1) * DK)
                        tmp = vp_pool.tile(
                            [128, JC, DK], F16, tag="vtmp", bufs=2, name="vtmp"
                        )
                        nc.scalar.dma_start(
                            tmp[:], VT[hs, b * 2048 : (b + 1) * 2048],
                            transpose=True,
                        )
                        vpt = vp_pool.tile(
                            [128, JC, DK + 1], F16, tag=f"vp_{h}_{b}",
                            name=f"vp_{h}_{b}",
                        )
                        nc.gpsimd.memset(vpt[:, :, DK : DK + 1], 1.0)
                        nc.vector.tensor_copy(vpt[:, :, 0:DK], tmp[:])
                        vp_tiles[(h, b)] = vpt

                halves = []
                for b in range(BATCH):
                    for TT, w_sb, x_dram, t in (
                        (KT, wk_sb, kT, "k"),
                        (QT, wq_sb, qT, "q"),
                        (VT, wv_sb, vT, "v"),
                    ):
                        halves.append((TT, w_sb, x_dram, t, b))
                # reorder: K_b0, Q_b0 first (pre-emitted), then the rest
                halves = [halves[0], halves[1], halves[2]] + halves[3:]
                steps = []
                for TT, w_sb, x_dram, t, b in halves:
                    box = [None]
                    for k in range(KT_TILES):
                        steps.append(
                            lambda TT=TT, w_sb=w_sb, x_dram=x_dram, b=b, k=k,
                            box=box: emit_half_kstep(TT, w_sb, x_dram, b, k, box)
                        )
                    if t == "v":
                        steps.append(lambda b=b: emit_vplus(b))

                # pre-emit K_b0 + Q_b0 (16 k-steps)
                for s in steps[:16]:
                    s()
                rest = steps[16:]
                ri = 0

                def run_steps(n):
                    nonlocal ri
                    for _ in range(n):
                        if ri < len(rest):
                            rest[ri]()
                            ri += 1

                st0 = {"w": windows[0], "e": {}}
                for p in range(JC):
                    emit_scores(st0, p)
                    run_steps(2)

                st1 = {"w": windows[1], "e": {}}
                for p in range(8):
                    emit_scores(st1, p)
                    run_steps(1)
                run_steps(len(rest))

            with tc.tile_pool(name="pso", bufs=2, space="PSUM") as pso:
                for p in range(8, JC):
                    emit_scores(st1, p)
                    emit_av(st0, pso, AV1_CHUNKS[p])

                av_st, wo_st = st1, st0
                for L in range(2, N_WIN + 2):
                    sc_st = {"w": windows[L], "e": {}} if L < N_WIN else None
                    if wo_st is not None:
                        emit_normalize(wo_st)
                    for p in range(JC):
                        if sc_st is not None:
                            emit_scores(sc_st, p)
                        if av_st is not None:
                            emit_av(av_st, pso, AV_CHUNKS[p])
                        if wo_st is not None and p in WO_POS:
                            emit_wo(wo_st, WO_POS[p])
                    wo_st, av_st = av_st, sc_st

    nc.compile()
    return nc


def kernel(q, k, v, Wq, Wk, Wv, Wo):
    q = np.asarray(q, dtype=np.float32)
    k = np.asarray(k, dtype=np.float32)
    v = np.asarray(v, dtype=np.float32)
    Wq = np.asarray(Wq, dtype=np.float32)
    Wk = np.asarray(Wk, dtype=np.float32)
    Wv = np.asarray(Wv, dtype=np.float32)
    Wo = np.asarray(Wo, dtype=np.float32)

    qT = np.ascontiguousarray(q.reshape(M, D).T.astype(np.float16))
    kT = np.ascontiguousarray(k.reshape(M, D).T.astype(np.float16))
    vT = np.ascontiguousarray(v.reshape(M, D).T.astype(np.float16))

    in_maps = []
    for c in range(N_CORES):
        cs = slice(c * HG, (c + 1) * HG)
        in_maps.append(
            {
                "qT": qT,
                "kT": kT,
                "vT": vT,
                "wqT": np.ascontiguousarray(Wq[cs, :].T.astype(np.float16)),
                "wkT": np.ascontiguousarray(Wk[cs, :].T.astype(np.float16)),
                "wvT": np.ascontiguousarray(Wv[cs, :].T.astype(np.float16)),
                "woT": np.ascontiguousarray(Wo[:, cs].T),
            }
        )

    nc = build_bass()

    def run_once():
        res = run_bass_kernel_spmd(nc, in_maps, core_ids=list(range(N_CORES)))
        acc = res.results[0]["out"].astype(np.float32)
        for c in range(1, N_CORES):
            acc = acc + res.results[c]["out"]
        return acc

    acc = run_once()
    if not np.isfinite(acc).all():
        acc = run_once()  # guard against sporadic device flake
    return acc.reshape(BATCH, SEQ, D)
